# revision 18
# baseline (speedup 1.0000x reference)
"""Trainium2 Bass kernel for nn_DynamicShortConvolution.

Reference computation (per token t, channel d):
    h    = silu(x @ w1)                       # [T, H]
    flat = h @ w2 + b2                        # [T, D*W]
    k    = flat.reshape(T, D, W)
    out[t, d] = silu(sum_w k[t, d, w] * x[t - (W-1) + w, d])

Sharding: 8 cores, each one (batch, half-of-T) shard of 2048 tokens plus a
3-token left halo.  Per-core tensors are TRANSPOSED ([D, T], channels on SBUF
partitions) so the causal shift is a free-dim offset and both matmuls run
without on-device transposes.

Schedule (v4) based on measured per-op costs:
  - DMA order w1, b2, x(16 tiles), w2(8 chunks); mm1 is dt-OUTER so it
    overlaps the x load and finishes right after the last x tile lands.
  - w2 stored dt-major so mm2 group (dt,pi) needs only its own chunk.
  - mm2 elementwise, per 1024-token group, engine-balanced:
      DVE : stt taps 1,3 straight from PSUM (evac+bias+product in one op,
            1x but errata/alignment/contention-immune), TT products for
            taps 0,2 (bf16 2x), one pairwise add
      ACT : bias-evac taps 0,2 (FD1024; FD2048 measured slower), silu
      PE  : 3-term identity-matmul reduce (m0 + m2 + a13) accumulated in
            PSUM -- the tensor engine replaces the DVE/GPSIMD add tree
      GPS : nothing (its SBUF-port sharing slows concurrent DVE 2x ops)
  - acc reuses the k2 PSUM region (subtile deps) so everything fits in 8
    banks; 2-iteration software-pipeline skew keeps all queues stall-free.
"""

import numpy as np

# Problem constants (hardcoded per harness contract).
B, T, D, H, W = 4, 4096, 2048, 256, 4
HALO = W - 1
N_CORES = 8
TOK = (B * T) // N_CORES  # tokens per core = 2048


def _build_nc(tok, d, h, xstride):
    import concourse.bass as bass
    import concourse.bacc as bacc
    import concourse.mybir as mybir
    import concourse.tile as tile

    f32 = mybir.dt.float32
    bf16 = mybir.dt.bfloat16
    AF = mybir.ActivationFunctionType
    ALU = mybir.AluOpType

    n_dt = d // 128        # 16 d tiles
    n_hc = h // 128        # 2 h tiles
    P = 1024               # tokens per mm2 group
    n_pi = tok // P        # 2
    NG = n_dt * n_pi       # 32 groups

    nc = bacc.Bacc()

    # DRAM I/O (host-prepared layouts)
    xT = nc.declare_dram_parameter("xT", [n_dt, 128, xstride], bf16, isOutput=False)
    # w1d[p, dt*h + j] = w1[dt*128+p, j]
    w1d = nc.declare_dram_parameter("w1d", [128, n_dt * h], bf16, isOutput=False)
    # w2d[p, dt*1024 + hc*512 + w*128 + c] = w2[hc*128+p, (dt*128+c)*W + w]
    w2d = nc.declare_dram_parameter("w2d", [128, n_dt * 1024], bf16, isOutput=False)
    # b2d[p, dt*W + w] = b2[(dt*128+p)*W + w]
    b2d = nc.declare_dram_parameter("b2d", [128, n_dt * W], f32, isOutput=False)
    # identity for PE reduce matmuls
    idd = nc.declare_dram_parameter("idd", [128, 128], bf16, isOutput=False)
    outT = nc.declare_dram_parameter("outT", [n_dt, 128, tok], bf16, isOutput=True)

    with tile.TileContext(nc) as tc:
        with (
            tc.tile_pool(name="resident", bufs=1) as rpool,
            tc.tile_pool(name="work", bufs=3) as wpool,
            tc.tile_pool(name="psum", bufs=1, space="PSUM") as ppool,
        ):
            # ---- resident tiles ----
            xT_sb = rpool.tile([128, n_dt * xstride], bf16, tag="xT")
            w1_sb = rpool.tile([128, n_dt * h], bf16, tag="w1")
            w2_sb = rpool.tile([128, n_dt * 1024], bf16, tag="w2")
            b2_sb = rpool.tile([128, n_dt * W], f32, tag="b2")
            id_sb = rpool.tile([128, 128], bf16, tag="idd")
            hT_sb = rpool.tile([128, n_hc * tok], bf16, tag="hT")

            # ---- DMA issue order: w1 chunks interleaved with x tiles so
            # mm1's first bursts start as early as possible ----
            for dt in range(n_dt):
                if dt % 4 == 0:
                    c = dt // 4
                    nc.sync.dma_start(
                        w1_sb[:, c * 4 * h:(c + 1) * 4 * h],
                        w1d[:, c * 4 * h:(c + 1) * 4 * h])
                if dt == 12:
                    nc.sync.dma_start(b2_sb[:, :], b2d[:, :])
                    nc.sync.dma_start(id_sb[:, :], idd[:, :])
                nc.sync.dma_start(
                    xT_sb[:, dt * xstride:(dt + 1) * xstride], xT[dt])
            for c in range(8):  # 2 dt per chunk
                nc.sync.dma_start(
                    w2_sb[:, c * 2048:(c + 1) * 2048],
                    w2d[:, c * 2048:(c + 1) * 2048])

            def x_slice(dt, col, n):
                return xT_sb[:, dt * xstride + col: dt * xstride + col + n]

            # Four separate PSUM tiles (2 banks each = all 8 banks) for
            # fine-grained dependencies: t1=k1, t3=k3, t0=k0, t2=k2.
            # The group reduce (acc) time-shares t0 after evac0 reads it.
            t1 = ppool.tile([128, P], f32, tag="t1")
            t3 = ppool.tile([128, P], f32, tag="t3")
            t0 = ppool.tile([128, P], f32, tag="t0")
            t2 = ppool.tile([128, P], f32, tag="t2")
            mm1_dst = [t1, t3, t0, t2]

            # ---- mm1 (dt-outer): hT = silu(w1.T @ xT) ----
            # tcp-outer within each dt so the tcp0 chains (all group-0
            # needs) finish first in the final burst.
            for dt in range(n_dt):
                for tcp in range(2):
                    for hc in range(n_hc):
                        pt = mm1_dst[hc * 2 + tcp]
                        for half in range(2):
                            nc.tensor.matmul(
                                pt[:, half * 512:(half + 1) * 512],
                                w1_sb[:, dt * h + hc * 128:
                                      dt * h + hc * 128 + 128],
                                x_slice(dt, HALO + (tcp * 2 + half) * 512, 512),
                                start=(dt == 0), stop=(dt == n_dt - 1),
                            )
            # evac tcp0 tiles first: group 0 only needs hT tokens 0:1024
            for i in (0, 2, 1, 3):
                nc.scalar.activation(
                    hT_sb[:, i * P:(i + 1) * P], mm1_dst[i][:], AF.Silu)

            # ---- mm2 + conv + silu, software pipeline over 32 groups ----
            st = [None] * NG

            def bias(dt, w):
                return b2_sb[:, dt * W + w: dt * W + w + 1]

            def tap_mms(g, w, pt, c0):
                dt, pi = divmod(g, n_pi)
                j0 = pi * P
                for hc in range(n_hc):
                    for tcj in range(2):
                        nc.tensor.matmul(
                            pt[:, c0 + tcj * 512: c0 + (tcj + 1) * 512],
                            w2_sb[:, dt * 1024 + hc * 512 + w * 128:
                                  dt * 1024 + hc * 512 + w * 128 + 128],
                            hT_sb[:, hc * tok + j0 + tcj * 512:
                                  hc * tok + j0 + (tcj + 1) * 512],
                            start=(hc == 0), stop=(hc == n_hc - 1),
                        )

            def acc_tile(g):
                # group g's conv-sum accumulates into t0 (even) / t2 (odd)
                return t0 if g % 2 == 0 else t2

            for g in range(NG + 2):
                # ---- ACT: silu of group g-2's acc (written at the end of
                # iteration g-1) + output DMA ----
                if 0 <= g - 2 < NG and not st[g - 2].get("done"):
                    dt2, pi2 = divmod(g - 2, n_pi)
                    ot = wpool.tile([128, P], bf16, tag="ot", name=f"ot_{g-2}")
                    nc.scalar.activation(ot[:], acc_tile(g - 2)[:], AF.Silu)
                    nc.sync.dma_start(
                        outT[dt2, :, pi2 * P:(pi2 + 1) * P], ot[:])

                # ---- PE: tap matmuls for group g.  The tap whose tile will
                # receive acc(g-1) at the end of this iteration goes FIRST
                # (so its evac can finish before the reduce needs the banks);
                # the tap sharing silu(g-2)'s tile goes LAST. ----
                par1 = (g - 1) % 2  # reduce target this iter: 0->t0, 1->t2
                if g < NG:
                    dt, pi = divmod(g, n_pi)
                    j0 = pi * P
                    if g == 0:
                        # t1/t0 are evacuated (silu'd) first after mm1
                        tap_mms(g, 1, t1, 0)
                        tap_mms(g, 0, t0, 0)
                        tap_mms(g, 3, t3, 0)
                        tap_mms(g, 2, t2, 0)
                    elif par1 == 1:
                        tap_mms(g, 2, t2, 0)
                        tap_mms(g, 1, t1, 0)
                        tap_mms(g, 3, t3, 0)
                        tap_mms(g, 0, t0, 0)
                    else:
                        tap_mms(g, 0, t0, 0)
                        tap_mms(g, 1, t1, 0)
                        tap_mms(g, 3, t3, 0)
                        tap_mms(g, 2, t2, 0)

                # ---- ACT: bias-evacs of group g (reduce-target tap first) --
                if g < NG:
                    kb = wpool.tile([128, 3 * P], bf16, tag="kb", name=f"kb_{g}")
                    ev0 = (kb[:, 0:P], t0, bias(dt, 0))
                    ev2 = (kb[:, P:2 * P], t2, bias(dt, 2))
                    ev1 = (kb[:, 2 * P:3 * P], t1, bias(dt, 1))
                    order = (ev2, ev1, ev0) if par1 == 1 else (ev0, ev1, ev2)
                    for dst, src, b in order:
                        nc.scalar.add(dst, src[:], b)
                    st[g] = dict(dt=dt, j0=j0, kb=kb)

                # ---- DVE: products m0/m2/m1 + a13 (g-1), stt tap 3 (g) ----
                if 0 <= g - 1 < NG:
                    s1 = st[g - 1]
                    dt1, j1, kb1 = s1["dt"], s1["j0"], s1["kb"]
                    m0 = wpool.tile([128, P], bf16, tag="m0", name=f"m0_{g-1}")
                    nc.vector.tensor_mul(m0[:], kb1[:, 0:P],
                                         x_slice(dt1, j1 + 0, P))
                    m2 = wpool.tile([128, P], bf16, tag="m2", name=f"m2_{g-1}")
                    nc.vector.tensor_mul(m2[:], kb1[:, P:2 * P],
                                         x_slice(dt1, j1 + 2, P))
                    a02 = wpool.tile([128, P], bf16, tag="a02", name=f"a02_{g-1}")
                    nc.vector.tensor_add(a02[:], m0[:], m2[:])
                    m1 = wpool.tile([128, P], bf16, tag="m1", name=f"m1_{g-1}")
                    nc.vector.tensor_mul(m1[:], kb1[:, 2 * P:3 * P],
                                         x_slice(dt1, j1 + 1, P))
                    a13 = wpool.tile([128, P], bf16, tag="a13", name=f"a13_{g-1}")
                    nc.vector.tensor_add(a13[:], m1[:], s1["m3"][:])
                    s1["a02"], s1["a13"] = a02, a13
                    if g - 1 == NG - 1:
                        # last group: finish on DVE + ACT directly, skipping
                        # the PE reduce and two pipeline drain periods
                        accf = wpool.tile([128, P], bf16, tag="accf",
                                          name="accf")
                        nc.vector.tensor_add(accf[:], a02[:], a13[:])
                        dtf, pif = divmod(g - 1, n_pi)
                        otf = wpool.tile([128, P], bf16, tag="ot",
                                         name="ot_last")
                        nc.scalar.activation(otf[:], accf[:], AF.Silu)
                        nc.sync.dma_start(
                            outT[dtf, :, pif * P:(pif + 1) * P], otf[:])
                        s1["done"] = True
                if g < NG:
                    m3 = wpool.tile([128, P], bf16, tag="m3", name=f"m3_{g}")
                    nc.vector.scalar_tensor_tensor(
                        m3[:], t3[:], bias(dt, 3),
                        x_slice(dt, j0 + 3, P), op0=ALU.add, op1=ALU.mult)
                    st[g]["m3"] = m3

                # ---- PE: 2-term identity reduce for group g-1 into its
                # parity tile (freed by that tap's evac above) ----
                if 0 <= g - 1 < NG and not st[g - 1].get("done"):
                    s1 = st[g - 1]
                    tacc = acc_tile(g - 1)
                    for ci, term in enumerate((s1["a02"], s1["a13"])):
                        for c in range(2):
                            nc.tensor.matmul(
                                tacc[:, c * 512:(c + 1) * 512],
                                id_sb[:, :],
                                term[:, c * 512:(c + 1) * 512],
                                start=(ci == 0), stop=(ci == 1),
                            )
    nc.compile()
    return nc


def _prep_shards(x, w1, w2, b2, tok, d, h, halo, xstride):
    """Host-side shard prep. Returns list of per-core in_maps."""
    import ml_dtypes
    bf16 = ml_dtypes.bfloat16

    n_dt = d // 128
    b, t, _ = x.shape
    shards_per_batch = (b * t // tok) // b

    # w1d[p, dt*h + j] = w1[dt*128+p, j]
    w1_r = np.ascontiguousarray(
        w1.reshape(n_dt, 128, h).transpose(1, 0, 2).reshape(128, n_dt * h)
    ).astype(bf16)
    # w2d[p, dt*1024 + hc*512 + w*128 + c] = w2[hc*128+p, (dt*128+c)*W + w]
    w2_4d = w2.reshape(2, 128, d, W)              # [hc, p, dcol, w]
    w2_5d = w2_4d.reshape(2, 128, n_dt, 128, W)   # [hc, p, dt, c, w]
    w2_r = np.ascontiguousarray(
        w2_5d.transpose(1, 2, 0, 4, 3)            # [p, dt, hc, w, c]
        .reshape(128, n_dt * 1024)).astype(bf16)
    # b2d[p, dt*W + w] = b2[(dt*128+p)*W + w]
    b2_r = np.ascontiguousarray(
        b2.reshape(n_dt, 128, W).transpose(1, 0, 2).reshape(128, n_dt * W)
    ).astype(np.float32)
    id_r = np.eye(128, dtype=np.float32).astype(bf16)

    in_maps = []
    for core in range(N_CORES):
        bi, half = divmod(core, shards_per_batch)
        t0 = half * tok
        xh = np.zeros((tok + halo, d), np.float32)
        lo = max(t0 - halo, 0)
        xh[halo - (t0 - lo):] = x[bi, lo: t0 + tok]
        xTc = np.zeros((n_dt, 128, xstride), bf16)
        xTc[:, :, : tok + halo] = (
            xh.T.astype(bf16).reshape(n_dt, 128, tok + halo))
        in_maps.append({
            "xT": xTc, "w1d": w1_r, "w2d": w2_r, "b2d": b2_r, "idd": id_r})
    return in_maps


_NC_CACHE = {}


def kernel(x, w1, w2, b2, trace=False):
    from concourse.bass_utils import run_bass_kernel_spmd

    tok, d, h = TOK, D, H
    xstride = tok + HALO + 1  # even -> keeps bf16 4B alignment per dtile
    key = (tok, d, h)
    if key not in _NC_CACHE:
        _NC_CACHE[key] = _build_nc(tok, d, h, xstride)
    nc = _NC_CACHE[key]

    in_maps = _prep_shards(
        np.asarray(x, np.float32), np.asarray(w1, np.float32),
        np.asarray(w2, np.float32), np.asarray(b2, np.float32),
        tok, d, h, HALO, xstride)

    res = run_bass_kernel_spmd(nc, in_maps, core_ids=list(range(N_CORES)),
                               trace=trace)
    kernel.last_result = res

    shards_per_batch = (B * T // tok) // B
    out = np.empty((B, T, D), np.float32)
    for core in range(N_CORES):
        bi, half = divmod(core, shards_per_batch)
        oT = res.results[core]["outT"]  # [n_dt, 128, tok]
        out[bi, half * tok:(half + 1) * tok] = (
            oT.reshape(d, tok).T.astype(np.float32))
    return out


# revision 19
# speedup vs baseline: 1.0025x; 1.0025x over previous
"""Trainium2 Bass kernel for nn_DynamicShortConvolution.

Reference computation (per token t, channel d):
    h    = silu(x @ w1)                       # [T, H]
    flat = h @ w2 + b2                        # [T, D*W]
    k    = flat.reshape(T, D, W)
    out[t, d] = silu(sum_w k[t, d, w] * x[t - (W-1) + w, d])

Sharding: 8 cores, each one (batch, half-of-T) shard of 2048 tokens plus a
3-token left halo.  Per-core tensors are TRANSPOSED ([D, T], channels on SBUF
partitions) so the causal shift is a free-dim offset and both matmuls run
without on-device transposes.

Schedule (v4) based on measured per-op costs:
  - DMA order w1, b2, x(16 tiles), w2(8 chunks); mm1 is dt-OUTER so it
    overlaps the x load and finishes right after the last x tile lands.
  - w2 stored dt-major so mm2 group (dt,pi) needs only its own chunk.
  - mm2 elementwise, per 1024-token group, engine-balanced:
      DVE : stt taps 1,3 straight from PSUM (evac+bias+product in one op,
            1x but errata/alignment/contention-immune), TT products for
            taps 0,2 (bf16 2x), one pairwise add
      ACT : bias-evac taps 0,2 (FD1024; FD2048 measured slower), silu
      PE  : 3-term identity-matmul reduce (m0 + m2 + a13) accumulated in
            PSUM -- the tensor engine replaces the DVE/GPSIMD add tree
      GPS : nothing (its SBUF-port sharing slows concurrent DVE 2x ops)
  - acc reuses the k2 PSUM region (subtile deps) so everything fits in 8
    banks; 2-iteration software-pipeline skew keeps all queues stall-free.
"""

import numpy as np

# Problem constants (hardcoded per harness contract).
B, T, D, H, W = 4, 4096, 2048, 256, 4
HALO = W - 1
N_CORES = 8
TOK = (B * T) // N_CORES  # tokens per core = 2048


def _build_nc(tok, d, h, xstride):
    import concourse.bass as bass
    import concourse.bacc as bacc
    import concourse.mybir as mybir
    import concourse.tile as tile

    f32 = mybir.dt.float32
    bf16 = mybir.dt.bfloat16
    AF = mybir.ActivationFunctionType
    ALU = mybir.AluOpType

    n_dt = d // 128        # 16 d tiles
    n_hc = h // 128        # 2 h tiles
    P = 1024               # tokens per mm2 group
    n_pi = tok // P        # 2
    NG = n_dt * n_pi       # 32 groups

    nc = bacc.Bacc()

    # DRAM I/O (host-prepared layouts)
    xT = nc.declare_dram_parameter("xT", [n_dt, 128, xstride], bf16, isOutput=False)
    # w1d[p, dt*h + j] = w1[dt*128+p, j]
    w1d = nc.declare_dram_parameter("w1d", [128, n_dt * h], bf16, isOutput=False)
    # w2d[p, dt*1024 + hc*512 + w*128 + c] = w2[hc*128+p, (dt*128+c)*W + w]
    w2d = nc.declare_dram_parameter("w2d", [128, n_dt * 1024], bf16, isOutput=False)
    # b2d[p, dt*W + w] = b2[(dt*128+p)*W + w]
    b2d = nc.declare_dram_parameter("b2d", [128, n_dt * W], f32, isOutput=False)
    # identity for PE reduce matmuls
    idd = nc.declare_dram_parameter("idd", [128, 128], bf16, isOutput=False)
    outT = nc.declare_dram_parameter("outT", [n_dt, 128, tok], bf16, isOutput=True)

    with tile.TileContext(nc) as tc:
        with (
            tc.tile_pool(name="resident", bufs=1) as rpool,
            tc.tile_pool(name="work", bufs=3) as wpool,
            tc.tile_pool(name="psum", bufs=1, space="PSUM") as ppool,
        ):
            # ---- resident tiles ----
            xT_sb = rpool.tile([128, n_dt * xstride], bf16, tag="xT")
            w1_sb = rpool.tile([128, n_dt * h], bf16, tag="w1")
            w2_sb = rpool.tile([128, n_dt * 1024], bf16, tag="w2")
            b2_sb = rpool.tile([128, n_dt * W], f32, tag="b2")
            id_sb = rpool.tile([128, 128], bf16, tag="idd")
            hT_sb = rpool.tile([128, n_hc * tok], bf16, tag="hT")

            # ---- DMA issue order: w1 chunks interleaved with x tiles so
            # mm1's first bursts start as early as possible ----
            for dt in range(n_dt):
                if dt % 4 == 0:
                    c = dt // 4
                    nc.sync.dma_start(
                        w1_sb[:, c * 4 * h:(c + 1) * 4 * h],
                        w1d[:, c * 4 * h:(c + 1) * 4 * h])
                if dt == 12:
                    nc.sync.dma_start(b2_sb[:, :], b2d[:, :])
                    nc.sync.dma_start(id_sb[:, :], idd[:, :])
                nc.sync.dma_start(
                    xT_sb[:, dt * xstride:(dt + 1) * xstride], xT[dt])
            for c in range(8):  # 2 dt per chunk
                nc.sync.dma_start(
                    w2_sb[:, c * 2048:(c + 1) * 2048],
                    w2d[:, c * 2048:(c + 1) * 2048])

            def x_slice(dt, col, n):
                return xT_sb[:, dt * xstride + col: dt * xstride + col + n]

            # Four separate PSUM tiles (2 banks each = all 8 banks) for
            # fine-grained dependencies: t1=k1, t3=k3, t0=k0, t2=k2.
            # The group reduce (acc) time-shares t0 after evac0 reads it.
            t1 = ppool.tile([128, P], f32, tag="t1")
            t3 = ppool.tile([128, P], f32, tag="t3")
            t0 = ppool.tile([128, P], f32, tag="t0")
            t2 = ppool.tile([128, P], f32, tag="t2")
            mm1_dst = [t1, t3, t0, t2]

            # ---- mm1 (dt-outer): hT = silu(w1.T @ xT) ----
            # hc-outer keeps the stationary w1 slice for 4 consecutive
            # matmuls (LDWEIGHTS stays hidden).
            for dt in range(n_dt):
                for hc in range(n_hc):
                    for tcp in range(2):
                        pt = mm1_dst[hc * 2 + tcp]
                        for half in range(2):
                            nc.tensor.matmul(
                                pt[:, half * 512:(half + 1) * 512],
                                w1_sb[:, dt * h + hc * 128:
                                      dt * h + hc * 128 + 128],
                                x_slice(dt, HALO + (tcp * 2 + half) * 512, 512),
                                start=(dt == 0), stop=(dt == n_dt - 1),
                            )
            # evac tcp0 tiles first: group 0 only needs hT tokens 0:1024
            for i in (0, 2, 1, 3):
                nc.scalar.activation(
                    hT_sb[:, i * P:(i + 1) * P], mm1_dst[i][:], AF.Silu)

            # ---- mm2 + conv + silu, software pipeline over 32 groups ----
            st = [None] * NG

            def bias(dt, w):
                return b2_sb[:, dt * W + w: dt * W + w + 1]

            def tap_mms(g, w, pt, c0):
                dt, pi = divmod(g, n_pi)
                j0 = pi * P
                for hc in range(n_hc):
                    for tcj in range(2):
                        nc.tensor.matmul(
                            pt[:, c0 + tcj * 512: c0 + (tcj + 1) * 512],
                            w2_sb[:, dt * 1024 + hc * 512 + w * 128:
                                  dt * 1024 + hc * 512 + w * 128 + 128],
                            hT_sb[:, hc * tok + j0 + tcj * 512:
                                  hc * tok + j0 + (tcj + 1) * 512],
                            start=(hc == 0), stop=(hc == n_hc - 1),
                        )

            def acc_tile(g):
                # group g's conv-sum accumulates into t0 (even) / t2 (odd)
                return t0 if g % 2 == 0 else t2

            for g in range(NG + 2):
                # ---- ACT: silu of group g-2's acc (written at the end of
                # iteration g-1) + output DMA ----
                if 0 <= g - 2 < NG and not st[g - 2].get("done"):
                    dt2, pi2 = divmod(g - 2, n_pi)
                    ot = wpool.tile([128, P], bf16, tag="ot", name=f"ot_{g-2}")
                    nc.scalar.activation(ot[:], acc_tile(g - 2)[:], AF.Silu)
                    nc.sync.dma_start(
                        outT[dt2, :, pi2 * P:(pi2 + 1) * P], ot[:])

                # ---- PE: tap matmuls for group g.  The tap whose tile will
                # receive acc(g-1) at the end of this iteration goes FIRST
                # (so its evac can finish before the reduce needs the banks);
                # the tap sharing silu(g-2)'s tile goes LAST. ----
                par1 = (g - 1) % 2  # reduce target this iter: 0->t0, 1->t2
                if g < NG:
                    dt, pi = divmod(g, n_pi)
                    j0 = pi * P
                    if g == 0:
                        # t1/t0 are evacuated (silu'd) first after mm1
                        tap_mms(g, 1, t1, 0)
                        tap_mms(g, 0, t0, 0)
                        tap_mms(g, 3, t3, 0)
                        tap_mms(g, 2, t2, 0)
                    elif par1 == 1:
                        tap_mms(g, 2, t2, 0)
                        tap_mms(g, 1, t1, 0)
                        tap_mms(g, 3, t3, 0)
                        tap_mms(g, 0, t0, 0)
                    else:
                        tap_mms(g, 0, t0, 0)
                        tap_mms(g, 1, t1, 0)
                        tap_mms(g, 3, t3, 0)
                        tap_mms(g, 2, t2, 0)

                # ---- ACT: bias-evacs of group g (reduce-target tap first) --
                if g < NG:
                    kb = wpool.tile([128, 3 * P], bf16, tag="kb", name=f"kb_{g}")
                    ev0 = (kb[:, 0:P], t0, bias(dt, 0))
                    ev2 = (kb[:, P:2 * P], t2, bias(dt, 2))
                    ev1 = (kb[:, 2 * P:3 * P], t1, bias(dt, 1))
                    order = (ev2, ev1, ev0) if par1 == 1 else (ev0, ev1, ev2)
                    for dst, src, b in order:
                        nc.scalar.add(dst, src[:], b)
                    st[g] = dict(dt=dt, j0=j0, kb=kb)

                # ---- DVE: products m0/m2/m1 + a13 (g-1), stt tap 3 (g) ----
                if 0 <= g - 1 < NG:
                    s1 = st[g - 1]
                    dt1, j1, kb1 = s1["dt"], s1["j0"], s1["kb"]
                    m0 = wpool.tile([128, P], bf16, tag="m0", name=f"m0_{g-1}")
                    nc.vector.tensor_mul(m0[:], kb1[:, 0:P],
                                         x_slice(dt1, j1 + 0, P))
                    m2 = wpool.tile([128, P], bf16, tag="m2", name=f"m2_{g-1}")
                    nc.vector.tensor_mul(m2[:], kb1[:, P:2 * P],
                                         x_slice(dt1, j1 + 2, P))
                    a02 = wpool.tile([128, P], bf16, tag="a02", name=f"a02_{g-1}")
                    nc.vector.tensor_add(a02[:], m0[:], m2[:])
                    m1 = wpool.tile([128, P], bf16, tag="m1", name=f"m1_{g-1}")
                    nc.vector.tensor_mul(m1[:], kb1[:, 2 * P:3 * P],
                                         x_slice(dt1, j1 + 1, P))
                    a13 = wpool.tile([128, P], bf16, tag="a13", name=f"a13_{g-1}")
                    nc.vector.tensor_add(a13[:], m1[:], s1["m3"][:])
                    s1["a02"], s1["a13"] = a02, a13
                    if g - 1 == NG - 1:
                        # last group: finish on DVE + ACT directly, skipping
                        # the PE reduce and two pipeline drain periods
                        accf = wpool.tile([128, P], bf16, tag="accf",
                                          name="accf")
                        nc.vector.tensor_add(accf[:], a02[:], a13[:])
                        dtf, pif = divmod(g - 1, n_pi)
                        otf = wpool.tile([128, P], bf16, tag="ot",
                                         name="ot_last")
                        nc.scalar.activation(otf[:], accf[:], AF.Silu)
                        nc.sync.dma_start(
                            outT[dtf, :, pif * P:(pif + 1) * P], otf[:])
                        s1["done"] = True
                if g < NG:
                    m3 = wpool.tile([128, P], bf16, tag="m3", name=f"m3_{g}")
                    nc.vector.scalar_tensor_tensor(
                        m3[:], t3[:], bias(dt, 3),
                        x_slice(dt, j0 + 3, P), op0=ALU.add, op1=ALU.mult)
                    st[g]["m3"] = m3

                # ---- PE: 2-term identity reduce for group g-1 into its
                # parity tile (freed by that tap's evac above) ----
                if 0 <= g - 1 < NG and not st[g - 1].get("done"):
                    s1 = st[g - 1]
                    tacc = acc_tile(g - 1)
                    for ci, term in enumerate((s1["a02"], s1["a13"])):
                        for c in range(2):
                            nc.tensor.matmul(
                                tacc[:, c * 512:(c + 1) * 512],
                                id_sb[:, :],
                                term[:, c * 512:(c + 1) * 512],
                                start=(ci == 0), stop=(ci == 1),
                            )
    nc.compile()
    return nc


def _prep_shards(x, w1, w2, b2, tok, d, h, halo, xstride):
    """Host-side shard prep. Returns list of per-core in_maps."""
    import ml_dtypes
    bf16 = ml_dtypes.bfloat16

    n_dt = d // 128
    b, t, _ = x.shape
    shards_per_batch = (b * t // tok) // b

    # w1d[p, dt*h + j] = w1[dt*128+p, j]
    w1_r = np.ascontiguousarray(
        w1.reshape(n_dt, 128, h).transpose(1, 0, 2).reshape(128, n_dt * h)
    ).astype(bf16)
    # w2d[p, dt*1024 + hc*512 + w*128 + c] = w2[hc*128+p, (dt*128+c)*W + w]
    w2_4d = w2.reshape(2, 128, d, W)              # [hc, p, dcol, w]
    w2_5d = w2_4d.reshape(2, 128, n_dt, 128, W)   # [hc, p, dt, c, w]
    w2_r = np.ascontiguousarray(
        w2_5d.transpose(1, 2, 0, 4, 3)            # [p, dt, hc, w, c]
        .reshape(128, n_dt * 1024)).astype(bf16)
    # b2d[p, dt*W + w] = b2[(dt*128+p)*W + w]
    b2_r = np.ascontiguousarray(
        b2.reshape(n_dt, 128, W).transpose(1, 0, 2).reshape(128, n_dt * W)
    ).astype(np.float32)
    id_r = np.eye(128, dtype=np.float32).astype(bf16)

    in_maps = []
    for core in range(N_CORES):
        bi, half = divmod(core, shards_per_batch)
        t0 = half * tok
        xh = np.zeros((tok + halo, d), np.float32)
        lo = max(t0 - halo, 0)
        xh[halo - (t0 - lo):] = x[bi, lo: t0 + tok]
        xTc = np.zeros((n_dt, 128, xstride), bf16)
        xTc[:, :, : tok + halo] = (
            xh.T.astype(bf16).reshape(n_dt, 128, tok + halo))
        in_maps.append({
            "xT": xTc, "w1d": w1_r, "w2d": w2_r, "b2d": b2_r, "idd": id_r})
    return in_maps


_NC_CACHE = {}


def kernel(x, w1, w2, b2, trace=False):
    from concourse.bass_utils import run_bass_kernel_spmd

    tok, d, h = TOK, D, H
    xstride = tok + HALO + 1  # even -> keeps bf16 4B alignment per dtile
    key = (tok, d, h)
    if key not in _NC_CACHE:
        _NC_CACHE[key] = _build_nc(tok, d, h, xstride)
    nc = _NC_CACHE[key]

    in_maps = _prep_shards(
        np.asarray(x, np.float32), np.asarray(w1, np.float32),
        np.asarray(w2, np.float32), np.asarray(b2, np.float32),
        tok, d, h, HALO, xstride)

    res = run_bass_kernel_spmd(nc, in_maps, core_ids=list(range(N_CORES)),
                               trace=trace)
    kernel.last_result = res

    shards_per_batch = (B * T // tok) // B
    out = np.empty((B, T, D), np.float32)
    for core in range(N_CORES):
        bi, half = divmod(core, shards_per_batch)
        oT = res.results[core]["outT"]  # [n_dt, 128, tok]
        out[bi, half * tok:(half + 1) * tok] = (
            oT.reshape(d, tok).T.astype(np.float32))
    return out


# revision 24
# speedup vs baseline: 1.0126x; 1.0101x over previous
"""Trainium2 Bass kernel for nn_DynamicShortConvolution.

Reference computation (per token t, channel d):
    h    = silu(x @ w1)                       # [T, H]
    flat = h @ w2 + b2                        # [T, D*W]
    k    = flat.reshape(T, D, W)
    out[t, d] = silu(sum_w k[t, d, w] * x[t - (W-1) + w, d])

Sharding: 8 cores, each one (batch, half-of-T) shard of 2048 tokens plus a
3-token left halo.  Per-core tensors are TRANSPOSED ([D, T], channels on SBUF
partitions) so the causal shift is a free-dim offset and both matmuls run
without on-device transposes.

Schedule (v4) based on measured per-op costs:
  - DMA order w1, b2, x(16 tiles), w2(8 chunks); mm1 is dt-OUTER so it
    overlaps the x load and finishes right after the last x tile lands.
  - w2 stored dt-major so mm2 group (dt,pi) needs only its own chunk.
  - mm2 elementwise, per 1024-token group, engine-balanced:
      DVE : stt taps 1,3 straight from PSUM (evac+bias+product in one op,
            1x but errata/alignment/contention-immune), TT products for
            taps 0,2 (bf16 2x), one pairwise add
      ACT : bias-evac taps 0,2 (FD1024; FD2048 measured slower), silu
      PE  : 3-term identity-matmul reduce (m0 + m2 + a13) accumulated in
            PSUM -- the tensor engine replaces the DVE/GPSIMD add tree
      GPS : nothing (its SBUF-port sharing slows concurrent DVE 2x ops)
  - acc reuses the k2 PSUM region (subtile deps) so everything fits in 8
    banks; 2-iteration software-pipeline skew keeps all queues stall-free.
"""

import numpy as np

# Problem constants (hardcoded per harness contract).
B, T, D, H, W = 4, 4096, 2048, 256, 4
HALO = W - 1
N_CORES = 8
TOK = (B * T) // N_CORES  # tokens per core = 2048


def _build_nc(tok, d, h, xstride):
    import concourse.bass as bass
    import concourse.bacc as bacc
    import concourse.mybir as mybir
    import concourse.tile as tile

    f32 = mybir.dt.float32
    bf16 = mybir.dt.bfloat16
    AF = mybir.ActivationFunctionType
    ALU = mybir.AluOpType

    n_dt = d // 128        # 16 d tiles
    n_hc = h // 128        # 2 h tiles
    P = 1024               # tokens per mm2 group
    n_pi = tok // P        # 2
    NG = n_dt * n_pi       # 32 groups

    nc = bacc.Bacc()

    # DRAM I/O (host-prepared layouts)
    xT = nc.declare_dram_parameter("xT", [n_dt, 128, xstride], bf16, isOutput=False)
    # w1d[p, dt*h + j] = w1[dt*128+p, j]
    w1d = nc.declare_dram_parameter("w1d", [128, n_dt * h], bf16, isOutput=False)
    # w2d[p, dt*1024 + hc*512 + w*128 + c] = w2[hc*128+p, (dt*128+c)*W + w]
    w2d = nc.declare_dram_parameter("w2d", [128, n_dt * 1024], bf16, isOutput=False)
    # b2d[p, dt*W + w] = b2[(dt*128+p)*W + w]
    b2d = nc.declare_dram_parameter("b2d", [128, n_dt * W], f32, isOutput=False)
    # identity for PE reduce matmuls
    idd = nc.declare_dram_parameter("idd", [128, 128], bf16, isOutput=False)
    outT = nc.declare_dram_parameter("outT", [n_dt, 128, tok], bf16, isOutput=True)

    with tile.TileContext(nc) as tc:
        with (
            tc.tile_pool(name="resident", bufs=1) as rpool,
            tc.tile_pool(name="work", bufs=3) as wpool,
            tc.tile_pool(name="psum", bufs=1, space="PSUM") as ppool,
        ):
            # ---- resident tiles ----
            xT_sb = rpool.tile([128, n_dt * xstride], bf16, tag="xT")
            w1_sb = rpool.tile([128, n_dt * h], bf16, tag="w1")
            w2_sb = rpool.tile([128, n_dt * 1024], bf16, tag="w2")
            b2_sb = rpool.tile([128, n_dt * W], f32, tag="b2")
            id_sb = rpool.tile([128, 128], bf16, tag="idd")
            hT_sb = rpool.tile([128, n_hc * tok], bf16, tag="hT")

            # ---- DMA issue order: w1 chunks interleaved with x tiles so
            # mm1's first bursts start as early as possible ----
            for dt in range(n_dt):
                if dt % 4 == 0:
                    c = dt // 4
                    nc.sync.dma_start(
                        w1_sb[:, c * 4 * h:(c + 1) * 4 * h],
                        w1d[:, c * 4 * h:(c + 1) * 4 * h])
                if dt == 12:
                    nc.sync.dma_start(b2_sb[:, :], b2d[:, :])
                    nc.sync.dma_start(id_sb[:, :], idd[:, :])
                nc.sync.dma_start(
                    xT_sb[:, dt * xstride:(dt + 1) * xstride], xT[dt])
            for c in range(8):  # 2 dt per chunk
                nc.sync.dma_start(
                    w2_sb[:, c * 2048:(c + 1) * 2048],
                    w2d[:, c * 2048:(c + 1) * 2048])

            def x_slice(dt, col, n):
                return xT_sb[:, dt * xstride + col: dt * xstride + col + n]

            # Four separate PSUM tiles (2 banks each = all 8 banks) for
            # fine-grained dependencies: t1=k1, t3=k3, t0=k0, t2=k2.
            # The group reduce (acc) time-shares t0 after evac0 reads it.
            t1 = ppool.tile([128, P], f32, tag="t1")
            t3 = ppool.tile([128, P], f32, tag="t3")
            t0 = ppool.tile([128, P], f32, tag="t0")
            t2 = ppool.tile([128, P], f32, tag="t2")
            mm1_dst = [t1, t3, t0, t2]

            # ---- mm1 (dt-outer): hT = silu(w1.T @ xT) ----
            # hc-outer keeps the stationary w1 slice for 4 consecutive
            # matmuls (LDWEIGHTS stays hidden).
            for dt in range(n_dt):
                for hc in range(n_hc):
                    for tcp in range(2):
                        pt = mm1_dst[hc * 2 + tcp]
                        for half in range(2):
                            nc.tensor.matmul(
                                pt[:, half * 512:(half + 1) * 512],
                                w1_sb[:, dt * h + hc * 128:
                                      dt * h + hc * 128 + 128],
                                x_slice(dt, HALO + (tcp * 2 + half) * 512, 512),
                                start=(dt == 0), stop=(dt == n_dt - 1),
                            )
            for i in range(4):
                nc.scalar.activation(
                    hT_sb[:, i * P:(i + 1) * P], mm1_dst[i][:], AF.Silu)

            # ---- mm2 + conv + silu, software pipeline over 32 groups ----
            st = [None] * NG

            def bias(dt, w):
                return b2_sb[:, dt * W + w: dt * W + w + 1]

            def tap_mms(g, w, pt, c0):
                dt, pi = divmod(g, n_pi)
                j0 = pi * P
                for hc in range(n_hc):
                    for tcj in range(2):
                        nc.tensor.matmul(
                            pt[:, c0 + tcj * 512: c0 + (tcj + 1) * 512],
                            w2_sb[:, dt * 1024 + hc * 512 + w * 128:
                                  dt * 1024 + hc * 512 + w * 128 + 128],
                            hT_sb[:, hc * tok + j0 + tcj * 512:
                                  hc * tok + j0 + (tcj + 1) * 512],
                            start=(hc == 0), stop=(hc == n_hc - 1),
                        )

            def acc_tile(g):
                # group g's conv-sum accumulates into t0 (even) / t2 (odd)
                return t0 if g % 2 == 0 else t2

            for g in range(NG + 2):
                # ---- ACT: silu of group g-2's acc (written at the end of
                # iteration g-1) + output DMA ----
                if 0 <= g - 2 < NG:
                    dt2, pi2 = divmod(g - 2, n_pi)
                    ot = wpool.tile([128, P], bf16, tag="ot", name=f"ot_{g-2}")
                    nc.scalar.activation(ot[:], acc_tile(g - 2)[:], AF.Silu)
                    nc.sync.dma_start(
                        outT[dt2, :, pi2 * P:(pi2 + 1) * P], ot[:])

                # ---- PE: tap matmuls for group g.  The tap whose tile will
                # receive acc(g-1) at the end of this iteration goes FIRST
                # (so its evac can finish before the reduce needs the banks);
                # the tap sharing silu(g-2)'s tile goes LAST. ----
                par1 = (g - 1) % 2  # reduce target this iter: 0->t0, 1->t2
                if g < NG:
                    dt, pi = divmod(g, n_pi)
                    j0 = pi * P
                    if par1 == 1:
                        tap_mms(g, 2, t2, 0)
                        tap_mms(g, 1, t1, 0)
                        tap_mms(g, 3, t3, 0)
                        tap_mms(g, 0, t0, 0)
                    else:
                        tap_mms(g, 0, t0, 0)
                        tap_mms(g, 1, t1, 0)
                        tap_mms(g, 3, t3, 0)
                        tap_mms(g, 2, t2, 0)

                # ---- ACT: bias-evacs of group g (reduce-target tap first) --
                if g < NG:
                    kb = wpool.tile([128, 3 * P], bf16, tag="kb", name=f"kb_{g}")
                    ev0 = (kb[:, 0:P], t0, bias(dt, 0))
                    ev2 = (kb[:, P:2 * P], t2, bias(dt, 2))
                    ev1 = (kb[:, 2 * P:3 * P], t1, bias(dt, 1))
                    order = (ev2, ev1, ev0) if par1 == 1 else (ev0, ev1, ev2)
                    for dst, src, b in order:
                        nc.scalar.add(dst, src[:], b)
                    st[g] = dict(dt=dt, j0=j0, kb=kb)

                # ---- DVE: products m0/m2/m1 + a13 (g-1), stt tap 3 (g) ----
                if 0 <= g - 1 < NG:
                    s1 = st[g - 1]
                    dt1, j1, kb1 = s1["dt"], s1["j0"], s1["kb"]
                    m0 = wpool.tile([128, P], bf16, tag="m0", name=f"m0_{g-1}")
                    nc.vector.tensor_mul(m0[:], kb1[:, 0:P],
                                         x_slice(dt1, j1 + 0, P))
                    m2 = wpool.tile([128, P], bf16, tag="m2", name=f"m2_{g-1}")
                    nc.vector.tensor_mul(m2[:], kb1[:, P:2 * P],
                                         x_slice(dt1, j1 + 2, P))
                    a02 = wpool.tile([128, P], bf16, tag="a02", name=f"a02_{g-1}")
                    nc.vector.tensor_add(a02[:], m0[:], m2[:])
                    m1 = wpool.tile([128, P], bf16, tag="m1", name=f"m1_{g-1}")
                    nc.vector.tensor_mul(m1[:], kb1[:, 2 * P:3 * P],
                                         x_slice(dt1, j1 + 1, P))
                    a13 = wpool.tile([128, P], bf16, tag="a13", name=f"a13_{g-1}")
                    nc.vector.tensor_add(a13[:], m1[:], s1["m3"][:])
                    s1["a02"], s1["a13"] = a02, a13
                if g < NG:
                    m3 = wpool.tile([128, P], bf16, tag="m3", name=f"m3_{g}")
                    nc.vector.scalar_tensor_tensor(
                        m3[:], t3[:], bias(dt, 3),
                        x_slice(dt, j0 + 3, P), op0=ALU.add, op1=ALU.mult)
                    st[g]["m3"] = m3

                # ---- PE: 2-term identity reduce for group g-1 into its
                # parity tile (freed by that tap's evac above) ----
                if 0 <= g - 1 < NG:
                    s1 = st[g - 1]
                    tacc = acc_tile(g - 1)
                    for ci, term in enumerate((s1["a02"], s1["a13"])):
                        for c in range(2):
                            nc.tensor.matmul(
                                tacc[:, c * 512:(c + 1) * 512],
                                id_sb[:, :],
                                term[:, c * 512:(c + 1) * 512],
                                start=(ci == 0), stop=(ci == 1),
                            )
    nc.compile()
    return nc


def _prep_shards(x, w1, w2, b2, tok, d, h, halo, xstride):
    """Host-side shard prep. Returns list of per-core in_maps."""
    import ml_dtypes
    bf16 = ml_dtypes.bfloat16

    n_dt = d // 128
    b, t, _ = x.shape
    shards_per_batch = (b * t // tok) // b

    # w1d[p, dt*h + j] = w1[dt*128+p, j]
    w1_r = np.ascontiguousarray(
        w1.reshape(n_dt, 128, h).transpose(1, 0, 2).reshape(128, n_dt * h)
    ).astype(bf16)
    # w2d[p, dt*1024 + hc*512 + w*128 + c] = w2[hc*128+p, (dt*128+c)*W + w]
    w2_4d = w2.reshape(2, 128, d, W)              # [hc, p, dcol, w]
    w2_5d = w2_4d.reshape(2, 128, n_dt, 128, W)   # [hc, p, dt, c, w]
    w2_r = np.ascontiguousarray(
        w2_5d.transpose(1, 2, 0, 4, 3)            # [p, dt, hc, w, c]
        .reshape(128, n_dt * 1024)).astype(bf16)
    # b2d[p, dt*W + w] = b2[(dt*128+p)*W + w]
    b2_r = np.ascontiguousarray(
        b2.reshape(n_dt, 128, W).transpose(1, 0, 2).reshape(128, n_dt * W)
    ).astype(np.float32)
    id_r = np.eye(128, dtype=np.float32).astype(bf16)

    in_maps = []
    for core in range(N_CORES):
        bi, half = divmod(core, shards_per_batch)
        t0 = half * tok
        xh = np.zeros((tok + halo, d), np.float32)
        lo = max(t0 - halo, 0)
        xh[halo - (t0 - lo):] = x[bi, lo: t0 + tok]
        xTc = np.zeros((n_dt, 128, xstride), bf16)
        xTc[:, :, : tok + halo] = (
            xh.T.astype(bf16).reshape(n_dt, 128, tok + halo))
        in_maps.append({
            "xT": xTc, "w1d": w1_r, "w2d": w2_r, "b2d": b2_r, "idd": id_r})
    return in_maps


_NC_CACHE = {}


def kernel(x, w1, w2, b2, trace=False):
    from concourse.bass_utils import run_bass_kernel_spmd

    tok, d, h = TOK, D, H
    xstride = tok + HALO + 1  # even -> keeps bf16 4B alignment per dtile
    key = (tok, d, h)
    if key not in _NC_CACHE:
        _NC_CACHE[key] = _build_nc(tok, d, h, xstride)
    nc = _NC_CACHE[key]

    in_maps = _prep_shards(
        np.asarray(x, np.float32), np.asarray(w1, np.float32),
        np.asarray(w2, np.float32), np.asarray(b2, np.float32),
        tok, d, h, HALO, xstride)

    res = run_bass_kernel_spmd(nc, in_maps, core_ids=list(range(N_CORES)),
                               trace=trace)
    kernel.last_result = res

    shards_per_batch = (B * T // tok) // B
    out = np.empty((B, T, D), np.float32)
    for core in range(N_CORES):
        bi, half = divmod(core, shards_per_batch)
        oT = res.results[core]["outT"]  # [n_dt, 128, tok]
        out[bi, half * tok:(half + 1) * tok] = (
            oT.reshape(d, tok).T.astype(np.float32))
    return out


# revision 29
# speedup vs baseline: 1.0143x; 1.0016x over previous
"""Trainium2 Bass kernel for nn_DynamicShortConvolution.

Reference computation (per token t, channel d):
    h    = silu(x @ w1)                       # [T, H]
    flat = h @ w2 + b2                        # [T, D*W]
    k    = flat.reshape(T, D, W)
    out[t, d] = silu(sum_w k[t, d, w] * x[t - (W-1) + w, d])

Sharding: 8 cores, each one (batch, half-of-T) shard of 2048 tokens plus a
3-token left halo.  Per-core tensors are TRANSPOSED ([D, T], channels on SBUF
partitions) so the causal shift is a free-dim offset and both matmuls run
without on-device transposes.

Schedule (v4) based on measured per-op costs:
  - DMA order w1, b2, x(16 tiles), w2(8 chunks); mm1 is dt-OUTER so it
    overlaps the x load and finishes right after the last x tile lands.
  - w2 stored dt-major so mm2 group (dt,pi) needs only its own chunk.
  - mm2 elementwise, per 1024-token group, engine-balanced:
      DVE : stt taps 1,3 straight from PSUM (evac+bias+product in one op,
            1x but errata/alignment/contention-immune), TT products for
            taps 0,2 (bf16 2x), one pairwise add
      ACT : bias-evac taps 0,2 (FD1024; FD2048 measured slower), silu
      PE  : 3-term identity-matmul reduce (m0 + m2 + a13) accumulated in
            PSUM -- the tensor engine replaces the DVE/GPSIMD add tree
      GPS : nothing (its SBUF-port sharing slows concurrent DVE 2x ops)
  - acc reuses the k2 PSUM region (subtile deps) so everything fits in 8
    banks; 2-iteration software-pipeline skew keeps all queues stall-free.
"""

import numpy as np

# Problem constants (hardcoded per harness contract).
B, T, D, H, W = 4, 4096, 2048, 256, 4
HALO = W - 1
N_CORES = 8
TOK = (B * T) // N_CORES  # tokens per core = 2048


def _build_nc(tok, d, h, xstride):
    import concourse.bass as bass
    import concourse.bacc as bacc
    import concourse.mybir as mybir
    import concourse.tile as tile

    f32 = mybir.dt.float32
    bf16 = mybir.dt.bfloat16
    AF = mybir.ActivationFunctionType
    ALU = mybir.AluOpType

    n_dt = d // 128        # 16 d tiles
    n_hc = h // 128        # 2 h tiles
    P = 1024               # tokens per mm2 group
    n_pi = tok // P        # 2
    NG = n_dt * n_pi       # 32 groups

    nc = bacc.Bacc()

    # DRAM I/O (host-prepared layouts)
    xT = nc.declare_dram_parameter("xT", [n_dt, 128, xstride], bf16, isOutput=False)
    # w1d[p, dt*h + j] = w1[dt*128+p, j]
    w1d = nc.declare_dram_parameter("w1d", [128, n_dt * h], bf16, isOutput=False)
    # w2d[p, dt*1024 + hc*512 + w*128 + c] = w2[hc*128+p, (dt*128+c)*W + w]
    w2d = nc.declare_dram_parameter("w2d", [128, n_dt * 1024], bf16, isOutput=False)
    # b2d[p, dt*W + w] = b2[(dt*128+p)*W + w]
    b2d = nc.declare_dram_parameter("b2d", [128, n_dt * W], f32, isOutput=False)
    # identity for PE reduce matmuls
    idd = nc.declare_dram_parameter("idd", [128, 128], bf16, isOutput=False)
    outT = nc.declare_dram_parameter("outT", [n_dt, 128, tok], bf16, isOutput=True)

    with tile.TileContext(nc) as tc:
        with (
            tc.tile_pool(name="resident", bufs=1) as rpool,
            tc.tile_pool(name="work", bufs=3) as wpool,
            tc.tile_pool(name="psum", bufs=1, space="PSUM") as ppool,
        ):
            # ---- resident tiles ----
            xT_sb = rpool.tile([128, n_dt * xstride], bf16, tag="xT")
            w1_sb = rpool.tile([128, n_dt * h], bf16, tag="w1")
            w2_sb = rpool.tile([128, n_dt * 1024], bf16, tag="w2")
            b2_sb = rpool.tile([128, n_dt * W], f32, tag="b2")
            id_sb = rpool.tile([128, 128], bf16, tag="idd")
            hT_sb = rpool.tile([128, n_hc * tok], bf16, tag="hT")

            # ---- DMA issue order: w1 chunks interleaved with x tiles so
            # mm1's first bursts start as early as possible ----
            for dt in range(n_dt):
                if dt % 4 == 0:
                    c = dt // 4
                    nc.sync.dma_start(
                        w1_sb[:, c * 4 * h:(c + 1) * 4 * h],
                        w1d[:, c * 4 * h:(c + 1) * 4 * h])
                if dt == 12:
                    nc.sync.dma_start(b2_sb[:, :], b2d[:, :])
                    nc.sync.dma_start(id_sb[:, :], idd[:, :])
                nc.sync.dma_start(
                    xT_sb[:, dt * xstride:(dt + 1) * xstride], xT[dt])
            for c in range(8):  # 2 dt per chunk
                nc.sync.dma_start(
                    w2_sb[:, c * 2048:(c + 1) * 2048],
                    w2d[:, c * 2048:(c + 1) * 2048])

            def x_slice(dt, col, n):
                return xT_sb[:, dt * xstride + col: dt * xstride + col + n]

            # Four separate PSUM tiles (2 banks each = all 8 banks) for
            # fine-grained dependencies: t1=k1, t3=k3, t0=k0, t2=k2.
            # The group reduce (acc) time-shares t0 after evac0 reads it.
            t1 = ppool.tile([128, P], f32, tag="t1")
            t3 = ppool.tile([128, P], f32, tag="t3")
            t0 = ppool.tile([128, P], f32, tag="t0")
            t2 = ppool.tile([128, P], f32, tag="t2")
            mm1_dst = [t1, t3, t0, t2]

            # ---- mm1 (dt-outer): hT = silu(w1.T @ xT) ----
            # hc-outer keeps the stationary w1 slice for 4 consecutive
            # matmuls (LDWEIGHTS stays hidden).
            for dt in range(n_dt):
                for hc in range(n_hc):
                    for tcp in range(2):
                        pt = mm1_dst[hc * 2 + tcp]
                        for half in range(2):
                            nc.tensor.matmul(
                                pt[:, half * 512:(half + 1) * 512],
                                w1_sb[:, dt * h + hc * 128:
                                      dt * h + hc * 128 + 128],
                                x_slice(dt, HALO + (tcp * 2 + half) * 512, 512),
                                start=(dt == 0), stop=(dt == n_dt - 1),
                            )
            for i in (0, 2, 1, 3):  # tcp0 tiles first: group 0 needs them
                nc.scalar.activation(
                    hT_sb[:, i * P:(i + 1) * P], mm1_dst[i][:], AF.Silu)

            # ---- mm2 + conv + silu, software pipeline over 32 groups ----
            st = [None] * NG

            def bias(dt, w):
                return b2_sb[:, dt * W + w: dt * W + w + 1]

            def tap_mms(g, w, pt, c0):
                dt, pi = divmod(g, n_pi)
                j0 = pi * P
                for hc in range(n_hc):
                    for tcj in range(2):
                        nc.tensor.matmul(
                            pt[:, c0 + tcj * 512: c0 + (tcj + 1) * 512],
                            w2_sb[:, dt * 1024 + hc * 512 + w * 128:
                                  dt * 1024 + hc * 512 + w * 128 + 128],
                            hT_sb[:, hc * tok + j0 + tcj * 512:
                                  hc * tok + j0 + (tcj + 1) * 512],
                            start=(hc == 0), stop=(hc == n_hc - 1),
                        )

            def acc_tile(g):
                # group g's conv-sum accumulates into t0 (even) / t2 (odd)
                return t0 if g % 2 == 0 else t2

            for g in range(NG + 2):
                # ---- ACT: silu of group g-2's acc (written at the end of
                # iteration g-1) + output DMA ----
                if 0 <= g - 2 < NG and not st[g - 2].get("done"):
                    dt2, pi2 = divmod(g - 2, n_pi)
                    ot = wpool.tile([128, P], bf16, tag="ot", name=f"ot_{g-2}")
                    nc.scalar.activation(ot[:], acc_tile(g - 2)[:], AF.Silu)
                    nc.sync.dma_start(
                        outT[dt2, :, pi2 * P:(pi2 + 1) * P], ot[:])

                # ---- PE: tap matmuls for group g.  The tap whose tile will
                # receive acc(g-1) at the end of this iteration goes FIRST
                # (so its evac can finish before the reduce needs the banks);
                # the tap sharing silu(g-2)'s tile goes LAST. ----
                par1 = (g - 1) % 2  # reduce target this iter: 0->t0, 1->t2
                if g < NG:
                    dt, pi = divmod(g, n_pi)
                    j0 = pi * P
                    if g == 0:
                        # t1/t0 freed first by the mm1 evac order above
                        tap_mms(g, 1, t1, 0)
                        tap_mms(g, 0, t0, 0)
                        tap_mms(g, 3, t3, 0)
                        tap_mms(g, 2, t2, 0)
                    elif par1 == 1:
                        tap_mms(g, 2, t2, 0)
                        tap_mms(g, 1, t1, 0)
                        tap_mms(g, 3, t3, 0)
                        tap_mms(g, 0, t0, 0)
                    else:
                        tap_mms(g, 0, t0, 0)
                        tap_mms(g, 1, t1, 0)
                        tap_mms(g, 3, t3, 0)
                        tap_mms(g, 2, t2, 0)

                # ---- ACT: bias-evacs of group g (reduce-target tap first) --
                if g < NG:
                    kb = wpool.tile([128, 3 * P], bf16, tag="kb", name=f"kb_{g}")
                    ev0 = (kb[:, 0:P], t0, bias(dt, 0))
                    ev2 = (kb[:, P:2 * P], t2, bias(dt, 2))
                    ev1 = (kb[:, 2 * P:3 * P], t1, bias(dt, 1))
                    order = (ev2, ev1, ev0) if par1 == 1 else (ev0, ev1, ev2)
                    for dst, src, b in order:
                        nc.scalar.add(dst, src[:], b)
                    st[g] = dict(dt=dt, j0=j0, kb=kb)

                # ---- DVE: products m0/m2/m1 + a13 (g-1), stt tap 3 (g) ----
                if 0 <= g - 1 < NG:
                    s1 = st[g - 1]
                    dt1, j1, kb1 = s1["dt"], s1["j0"], s1["kb"]
                    m0 = wpool.tile([128, P], bf16, tag="m0", name=f"m0_{g-1}")
                    nc.vector.tensor_mul(m0[:], kb1[:, 0:P],
                                         x_slice(dt1, j1 + 0, P))
                    m2 = wpool.tile([128, P], bf16, tag="m2", name=f"m2_{g-1}")
                    nc.vector.tensor_mul(m2[:], kb1[:, P:2 * P],
                                         x_slice(dt1, j1 + 2, P))
                    a02 = wpool.tile([128, P], bf16, tag="a02", name=f"a02_{g-1}")
                    nc.vector.tensor_add(a02[:], m0[:], m2[:])
                    m1 = wpool.tile([128, P], bf16, tag="m1", name=f"m1_{g-1}")
                    nc.vector.tensor_mul(m1[:], kb1[:, 2 * P:3 * P],
                                         x_slice(dt1, j1 + 1, P))
                    a13 = wpool.tile([128, P], bf16, tag="a13", name=f"a13_{g-1}")
                    nc.vector.tensor_add(a13[:], m1[:], s1["m3"][:])
                    s1["a02"], s1["a13"] = a02, a13
                    if g - 1 == NG - 1:
                        # last group: finish on DVE+ACT, skip 2 drain periods
                        accf = wpool.tile([128, P], bf16, tag="accf",
                                          name="accf")
                        nc.vector.tensor_add(accf[:], a02[:], a13[:])
                        dtf, pif = divmod(g - 1, n_pi)
                        otf = wpool.tile([128, P], bf16, tag="ot",
                                         name="ot_last")
                        nc.scalar.activation(otf[:], accf[:], AF.Silu)
                        nc.sync.dma_start(
                            outT[dtf, :, pif * P:(pif + 1) * P], otf[:])
                        s1["done"] = True
                if g < NG:
                    m3 = wpool.tile([128, P], bf16, tag="m3", name=f"m3_{g}")
                    nc.vector.scalar_tensor_tensor(
                        m3[:], t3[:], bias(dt, 3),
                        x_slice(dt, j0 + 3, P), op0=ALU.add, op1=ALU.mult)
                    st[g]["m3"] = m3

                # ---- PE: 2-term identity reduce for group g-1 into its
                # parity tile (freed by that tap's evac above) ----
                if 0 <= g - 1 < NG and not st[g - 1].get("done"):
                    s1 = st[g - 1]
                    tacc = acc_tile(g - 1)
                    for ci, term in enumerate((s1["a02"], s1["a13"])):
                        for c in range(2):
                            nc.tensor.matmul(
                                tacc[:, c * 512:(c + 1) * 512],
                                id_sb[:, :],
                                term[:, c * 512:(c + 1) * 512],
                                start=(ci == 0), stop=(ci == 1),
                            )
    nc.compile()
    return nc


def _prep_shards(x, w1, w2, b2, tok, d, h, halo, xstride):
    """Host-side shard prep. Returns list of per-core in_maps."""
    import ml_dtypes
    bf16 = ml_dtypes.bfloat16

    n_dt = d // 128
    b, t, _ = x.shape
    shards_per_batch = (b * t // tok) // b

    # w1d[p, dt*h + j] = w1[dt*128+p, j]
    w1_r = np.ascontiguousarray(
        w1.reshape(n_dt, 128, h).transpose(1, 0, 2).reshape(128, n_dt * h)
    ).astype(bf16)
    # w2d[p, dt*1024 + hc*512 + w*128 + c] = w2[hc*128+p, (dt*128+c)*W + w]
    w2_4d = w2.reshape(2, 128, d, W)              # [hc, p, dcol, w]
    w2_5d = w2_4d.reshape(2, 128, n_dt, 128, W)   # [hc, p, dt, c, w]
    w2_r = np.ascontiguousarray(
        w2_5d.transpose(1, 2, 0, 4, 3)            # [p, dt, hc, w, c]
        .reshape(128, n_dt * 1024)).astype(bf16)
    # b2d[p, dt*W + w] = b2[(dt*128+p)*W + w]
    b2_r = np.ascontiguousarray(
        b2.reshape(n_dt, 128, W).transpose(1, 0, 2).reshape(128, n_dt * W)
    ).astype(np.float32)
    id_r = np.eye(128, dtype=np.float32).astype(bf16)

    in_maps = []
    for core in range(N_CORES):
        bi, half = divmod(core, shards_per_batch)
        t0 = half * tok
        xh = np.zeros((tok + halo, d), np.float32)
        lo = max(t0 - halo, 0)
        xh[halo - (t0 - lo):] = x[bi, lo: t0 + tok]
        xTc = np.zeros((n_dt, 128, xstride), bf16)
        xTc[:, :, : tok + halo] = (
            xh.T.astype(bf16).reshape(n_dt, 128, tok + halo))
        in_maps.append({
            "xT": xTc, "w1d": w1_r, "w2d": w2_r, "b2d": b2_r, "idd": id_r})
    return in_maps


_NC_CACHE = {}


def kernel(x, w1, w2, b2, trace=False):
    from concourse.bass_utils import run_bass_kernel_spmd

    tok, d, h = TOK, D, H
    xstride = tok + HALO + 1  # even -> keeps bf16 4B alignment per dtile
    key = (tok, d, h)
    if key not in _NC_CACHE:
        _NC_CACHE[key] = _build_nc(tok, d, h, xstride)
    nc = _NC_CACHE[key]

    in_maps = _prep_shards(
        np.asarray(x, np.float32), np.asarray(w1, np.float32),
        np.asarray(w2, np.float32), np.asarray(b2, np.float32),
        tok, d, h, HALO, xstride)

    res = run_bass_kernel_spmd(nc, in_maps, core_ids=list(range(N_CORES)),
                               trace=trace)
    kernel.last_result = res

    shards_per_batch = (B * T // tok) // B
    out = np.empty((B, T, D), np.float32)
    for core in range(N_CORES):
        bi, half = divmod(core, shards_per_batch)
        oT = res.results[core]["outT"]  # [n_dt, 128, tok]
        out[bi, half * tok:(half + 1) * tok] = (
            oT.reshape(d, tok).T.astype(np.float32))
    return out
